# revision 1
# baseline (speedup 1.0000x reference)
"""DSTMamba Trainium2 kernel: 8 NeuronCores, SPMD.

Core c handles (batch b=c//2, direction d=c%2). Odd cores receive the
token axis (n) reversed so the same forward-scan program computes the
reverse-direction Mamba branch; the bidirectional merge is a pair
AllReduce + subtract-own-contribution + reversed copy (symmetric SPMD,
no control flow).

Device layouts are transposed: activations are [feature, time] tiles so
every matmul takes host-pre-transposed lhsT weights (float32r = TF32ish
full-rate) and the Mamba recurrence is tensor_tensor_scan along the
free/time axis. dA factors are generated by ScalarE as exp(-(s+1)*dt)
straight into PSUM; dBx / C-products run on VectorE in bf16 at 2x.
"""

import numpy as np

import concourse.bacc as bacc
import concourse.mybir as mybir
from concourse import tile
from concourse.bass_utils import run_bass_kernel_spmd

B, L, H, N = 4, 512, 96, 862
DM, DS = 256, 16
DI = 512
DTR = 16
DFF, NLAYERS = 256, 2
DSL, KSTD = 3, 25
EPS = 1e-5

F32 = mybir.dt.float32
F32R = mybir.dt.float32r
BF16 = mybir.dt.bfloat16
AL = mybir.AluOpType
AF = mybir.ActivationFunctionType

NC2 = [(0, 512), (512, 350)]  # even moving-dim chunks covering N=862
PAIRS = [[0, 1], [2, 3], [4, 5], [6, 7]]

DEBUG = False
_CACHE = {}


# ---------------------------------------------------------------- host math
def _mavg_matrix(length):
    M = np.zeros((length, length), np.float64)
    p = (KSTD - 1) // 2
    for i in range(length):
        for d in range(-p, p + 1):
            j = min(max(i + d, 0), length - 1)
            M[i, j] += 1.0 / KSTD
    return M


def _pool_matrix(lo, hi):
    P = np.zeros((lo, hi), np.float64)
    for i in range(lo):
        P[i, 2 * i] = 0.5
        P[i, 2 * i + 1] = 0.5
    return P


_TREND_OPS_CACHE = []


def _trend_ops():
    if _TREND_OPS_CACHE:
        return _TREND_OPS_CACHE
    ops = []
    P = np.eye(L)
    cur = L
    for s in range(DSL + 1):
        ops.append(_mavg_matrix(cur) @ P)
        if s < DSL:
            P = _pool_matrix(cur // 2, cur) @ P
            cur //= 2
    _TREND_OPS_CACHE.extend(ops)
    return ops  # [512,512],[256,512],[128,512],[64,512]


def _col(v):
    v = np.asarray(v, np.float32).reshape(-1)
    if v.size <= 128:
        return np.ascontiguousarray(v.reshape(-1, 1))
    return np.ascontiguousarray(v.reshape(-1, 128).T)


def _row(v):
    return np.ascontiguousarray(np.asarray(v, np.float32).reshape(1, -1))


def _t(m):
    return np.ascontiguousarray(np.asarray(m, np.float32).T)


def make_core_inputs(inputs, core):
    b, d = core // 2, core % 2
    g = lambda k: np.asarray(inputs[k], np.float32)

    m = {}
    x = g("history_data")[b, :, :, 0]
    if d == 1:
        x = x[:, ::-1]
    m["x_in"] = np.ascontiguousarray(x)

    tops = _trend_ops()
    m["seaop_T"] = _t(np.eye(L) - tops[0])
    for s in range(4):
        m[f"trop{s}_T"] = _t(tops[s])

    m["emb_lhsT"] = _t(g("emb_w"))
    m["emb_b"] = _col(g("emb_b"))

    for l in range(NLAYERS):
        m[f"in_lhsT_{l}"] = _t(g("m_in")[l, d])
        m[f"cw0_{l}"] = _col(g("m_conv_w")[l, d, :, 0])
        m[f"cw1_{l}"] = _col(g("m_conv_w")[l, d, :, 1])
        m[f"cb_{l}"] = _col(g("m_conv_b")[l, d])
        xpt = _t(g("m_xproj")[l, d])
        m[f"xpbc_lhsT_{l}"] = np.ascontiguousarray(xpt[:, DTR:])
        m[f"xpdt_lhsT_{l}"] = np.ascontiguousarray(xpt[:, :DTR])
        m[f"dt_lhsT_{l}"] = _t(g("m_dt_w")[l, d])
        m[f"dtb_{l}"] = _col(g("m_dt_b")[l, d])
        m[f"D_{l}"] = _col(g("m_D")[l, d])
        m[f"out_lhsT_{l}"] = _t(g("m_out")[l, d])
        m[f"n1w_{l}"] = _col(g("n1_w")[l])
        m[f"n1b_{l}"] = _col(g("n1_b")[l])
        m[f"n2w_{l}"] = _col(g("n2_w")[l])
        m[f"n2b_{l}"] = _col(g("n2_b")[l])
        m[f"f1_lhsT_{l}"] = _t(g("f1_w")[l])
        m[f"f1b_{l}"] = _col(g("f1_b")[l])
        m[f"f2_lhsT_{l}"] = _t(g("f2_w")[l])
        m[f"f2b_{l}"] = _col(g("f2_b")[l])

    m["encnw"] = _col(g("encn_w"))
    m["encnb"] = _col(g("encn_b"))
    m["proj_lhsT"] = _t(g("proj_w"))
    m["projb"] = _col(g("proj_b"))

    for i in range(DSL):
        m[f"u{i}w1_lhsT"] = _t(g(f"u{i}w1"))
        m[f"u{i}b1"] = _col(g(f"u{i}b1"))
        m[f"u{i}w2_lhsT"] = _t(g(f"u{i}w2"))
        m[f"u{i}b2"] = _col(g(f"u{i}b2"))
    for s in range(4):
        m[f"map{s}_lhsT"] = _t(g(f"map{s}_w"))
    m["mapb"] = _col(sum(g(f"map{s}_b") for s in range(4)))

    rvw, rvb, trw = g("revin_w"), g("revin_b"), g("tre_w")
    if d == 1:
        rvw, rvb, trw = rvw[::-1], rvb[::-1], trw[::-1]
    m["rvw_row"] = _row(rvw)
    m["rvb_row"] = _row(rvb)
    m["trw_row"] = _row(trw)
    m["ones_col"] = np.ones((128, 1), np.float32)
    return m


# ------------------------------------------------------------- device build
class _Ctx:
    pass


def _build():
    nc = bacc.Bacc("TRN2", target_bir_lowering=False, debug=False,
                   num_devices=8)

    def din(name, shape, dt=F32):
        return nc.dram_tensor(name, list(shape), dt, kind="ExternalInput").ap()

    I = {}
    I["x_in"] = din("x_in", [L, N], F32R)
    I["seaop_T"] = din("seaop_T", [L, L], F32R)
    for s, ls in enumerate([512, 256, 128, 64]):
        I[f"trop{s}_T"] = din(f"trop{s}_T", [L, ls], F32R)
    I["emb_lhsT"] = din("emb_lhsT", [L, DM], F32R)
    I["emb_b"] = din("emb_b", [128, DM // 128])
    for l in range(NLAYERS):
        I[f"in_lhsT_{l}"] = din(f"in_lhsT_{l}", [DM, 2 * DI], F32R)
        for k in ["cw0", "cw1", "cb", "dtb", "D"]:
            I[f"{k}_{l}"] = din(f"{k}_{l}", [128, DI // 128])
        I[f"xpbc_lhsT_{l}"] = din(f"xpbc_lhsT_{l}", [DI, 2 * DS], F32R)
        I[f"xpdt_lhsT_{l}"] = din(f"xpdt_lhsT_{l}", [DI, DTR], F32R)
        I[f"dt_lhsT_{l}"] = din(f"dt_lhsT_{l}", [DTR, DI], F32R)
        I[f"out_lhsT_{l}"] = din(f"out_lhsT_{l}", [DI, DM], F32R)
        for k in ["n1w", "n1b", "n2w", "n2b", "f1b", "f2b"]:
            I[f"{k}_{l}"] = din(f"{k}_{l}", [128, DM // 128])
        I[f"f1_lhsT_{l}"] = din(f"f1_lhsT_{l}", [DM, DFF], F32R)
        I[f"f2_lhsT_{l}"] = din(f"f2_lhsT_{l}", [DFF, DM], F32R)
    I["encnw"] = din("encnw", [128, DM // 128])
    I["encnb"] = din("encnb", [128, DM // 128])
    I["proj_lhsT"] = din("proj_lhsT", [DM, H], F32R)
    I["projb"] = din("projb", [H, 1])
    for i, (li, lo) in enumerate([(64, 128), (128, 256), (256, 512)]):
        I[f"u{i}w1_lhsT"] = din(f"u{i}w1_lhsT", [li, lo], F32R)
        I[f"u{i}b1"] = din(f"u{i}b1", [min(lo, 128), max(1, lo // 128)])
        I[f"u{i}w2_lhsT"] = din(f"u{i}w2_lhsT", [lo, lo], F32R)
        I[f"u{i}b2"] = din(f"u{i}b2", [min(lo, 128), max(1, lo // 128)])
    for s, ls in enumerate([512, 256, 128, 64]):
        I[f"map{s}_lhsT"] = din(f"map{s}_lhsT", [ls, H], F32R)
    I["mapb"] = din("mapb", [H, 1])
    for k in ["rvw_row", "rvb_row", "trw_row"]:
        I[k] = din(k, [1, N])
    I["ones_col"] = din("ones_col", [128, 1], F32R)

    out_pred = nc.dram_tensor("pred", [H, N], F32, kind="ExternalOutput").ap()

    c = _Ctx()
    c.nc, c.I, c.out_pred = nc, I, out_pred
    c.dbg = {}
    with tile.TileContext(nc) as tc:
        c.tc = tc
        _emit(c)
    nc.compile()
    return nc


def _dbg(c, name, aps):
    if not DEBUG:
        return
    nc = c.nc
    rows = sum(a.shape[0] for a in aps)
    o = nc.dram_tensor(f"dbg_{name}", [rows, N], F32, kind="ExternalOutput").ap()
    r0 = 0
    for a in aps:
        r = a.shape[0]
        nc.sync.dma_start(o[r0:r0 + r, :], a.bitcast(F32))
        r0 += r
    c.dbg[name] = o


def _load(c, pool, key, tag=None):
    ap = c.I[key]
    t_ = pool.tile(list(ap.shape), ap.dtype, name=key, tag=tag or key)
    c.nc.sync.dma_start(t_[:, :], ap[:, :])
    return t_


def _load_tiles(c, pool, key, tag=None):
    ap = c.I[key]
    K, M = ap.shape
    out = []
    for ko in range(0, K, 128):
        rowt = []
        for mo in range(0, M, 128):
            kk, mm = min(128, K - ko), min(128, M - mo)
            t_ = pool.tile([kk, mm], F32R, name=f"{key}_{ko}_{mo}",
                           tag=f"{tag or key}_{ko}_{mo}")
            c.nc.sync.dma_start(t_[:, :], ap[ko:ko + kk, mo:mo + mm])
            rowt.append(t_)
        out.append(rowt)
    return out


def _bcast(c, pool, row_ap, parts, tag, via_dram=True):
    """broadcast [1,N] (sbuf or dram) row to [parts, N] f32 sbuf tile."""
    nc = c.nc
    if via_dram:
        d = c.dp.tile([1, N], F32, name=f"bd_{tag}", tag=f"bd_{tag}")
        nc.sync.dma_start(d[:, :], row_ap.bitcast(F32))
        src = d[:, :]
    else:
        src = row_ap.bitcast(F32)
    bt = pool.tile([parts, N], F32, name=f"bc_{tag}", tag=f"bc_{tag}")
    nc.sync.dma_start(bt[:, :], src.broadcast_to([parts, N]))
    return bt


def _matsum(c, psum, lhs_tiles, rhs_tiles, n0, nl):
    """psum += sum_k lhs_tiles[k].T @ rhs_tiles[k][:, n0:n0+nl]"""
    nc = c.nc
    kn = len(lhs_tiles)
    for k in range(kn):
        nc.tensor.matmul(psum[:, :], lhs_tiles[k][:, :],
                         rhs_tiles[k][:, n0:n0 + nl],
                         start=(k == 0), stop=(k == kn - 1))


def _layer_norm(c, scr, xin, wcol, bcol, outpool, outtag):
    """xin: 2 [128,N] f32r tiles -> 2 [128,N] f32r tiles (norm over 256)."""
    nc, pm = c.nc, c.pm
    mrow = scr.tile([1, N], F32, name=f"lnm_{outtag}", tag="ln_mrow")
    qrow = scr.tile([1, N], F32, name=f"lnq_{outtag}", tag="ln_qrow")
    for n0, nl in NC2:
        ps = pm.tile([1, nl], F32, name="lnps", tag="mm1")
        for mi in range(2):
            nc.tensor.matmul(ps[:, :], c.ones_col[:, :], xin[mi][:, n0:n0 + nl],
                             start=(mi == 0), stop=(mi == 1))
        nc.scalar.activation(mrow[:, n0:n0 + nl], ps[:, :], AF.Copy,
                             scale=1.0 / DM)
        ps2 = pm.tile([1, nl], F32, name="lnps2", tag="mm1")
        for mi in range(2):
            sq = scr.tile([128, N], F32R, name="lnsq", tag="sq", bufs=2)
            nc.scalar.activation(sq[:, n0:n0 + nl],
                                 xin[mi][:, n0:n0 + nl].bitcast(F32), AF.Square)
            nc.tensor.matmul(ps2[:, :], c.ones_col[:, :], sq[:, n0:n0 + nl],
                             start=(mi == 0), stop=(mi == 1))
        nc.scalar.activation(qrow[:, n0:n0 + nl], ps2[:, :], AF.Copy,
                             scale=1.0 / DM)
    tmp_ = scr.tile([1, N], F32, name=f"lnt_{outtag}", tag="ln_trow")
    nc.vector.tensor_mul(tmp_[:, :], mrow[:, :], mrow[:, :])
    nc.vector.tensor_sub(qrow[:, :], qrow[:, :], tmp_[:, :])
    nc.scalar.activation(qrow[:, :], qrow[:, :], AF.Ln, bias=c.epscol[:1, :])
    nc.scalar.activation(qrow[:, :], qrow[:, :], AF.Exp, scale=-0.5)
    mb = _bcast(c, scr, mrow[:, :], 128, "lnm")
    rb = _bcast(c, scr, qrow[:, :], 128, "lnr")
    out = []
    for mi in range(2):
        o = outpool.tile([128, N], F32R, name=f"{outtag}{mi}", tag=f"{outtag}{mi}")
        d1 = scr.tile([128, N], F32, name="lnd1", tag="d1", bufs=2)
        nc.vector.tensor_sub(d1[:, :], xin[mi][:, :].bitcast(F32), mb[:, :])
        nc.vector.tensor_mul(d1[:, :], d1[:, :], rb[:, :])
        nc.vector.tensor_scalar(o[:, :], d1[:, :],
                                wcol[:, mi:mi + 1],
                                bcol[:, mi:mi + 1], AL.mult, AL.add)
        out.append(o)
    return out


def _emit(c):
    nc, tc, I = c.nc, c.tc, c.I
    import contextlib
    with contextlib.ExitStack() as est:
        gp = est.enter_context(tc.tile_pool(name="glob", bufs=1))
        pm = est.enter_context(tc.tile_pool(name="pmm", bufs=2, space="PSUM"))
        dp = est.enter_context(tc.tile_pool(name="drm", bufs=1, space="DRAM"))
        c.gp, c.pm, c.dp = gp, pm, dp

        c.ones_col = _load(c, gp, "ones_col")
        epscol = gp.tile([128, 1], F32, name="epscol", tag="epscol")
        c.nc.gpsimd.memset(epscol[:, :], EPS)
        c.epscol = epscol
        r_mean = gp.tile([1, N], F32, name="r_mean", tag="r_mean")
        r_std = gp.tile([1, N], F32, name="r_std", tag="r_std")
        r_wr = gp.tile([1, N], F32, name="r_wr", tag="r_wr")
        r_sc = gp.tile([1, N], F32, name="r_sc", tag="r_sc")
        c.r_mean, c.r_sc = r_mean, r_sc

        # ======================================================== stage A+B
        with tc.tile_pool(name="front", bufs=1) as fp:
            r_msq = fp.tile([1, N], F32, name="r_msq", tag="r_msq")
            X = []
            for ci in range(4):
                t_ = fp.tile([128, N], F32R, name=f"xin{ci}", tag=f"xin{ci}")
                nc.sync.dma_start(t_[:, :], I["x_in"][ci * 128:(ci + 1) * 128, :])
                X.append(t_)
            for n0, nl in NC2:
                ps = pm.tile([1, nl], F32, name="rvs", tag="mm1")
                for ci in range(4):
                    nc.tensor.matmul(ps[:, :], c.ones_col[:, :],
                                     X[ci][:, n0:n0 + nl],
                                     start=(ci == 0), stop=(ci == 3))
                nc.scalar.activation(r_mean[:, n0:n0 + nl], ps[:, :],
                                     AF.Copy, scale=1.0 / L)
                ps2 = pm.tile([1, nl], F32, name="rvq", tag="mm1")
                for ci in range(4):
                    sq = fp.tile([128, N], F32R, name="rvsq", tag="sq", bufs=2)
                    nc.scalar.activation(sq[:, n0:n0 + nl],
                                         X[ci][:, n0:n0 + nl].bitcast(F32),
                                         AF.Square)
                    nc.tensor.matmul(ps2[:, :], c.ones_col[:, :],
                                     sq[:, n0:n0 + nl],
                                     start=(ci == 0), stop=(ci == 3))
                nc.scalar.activation(r_msq[:, n0:n0 + nl], ps2[:, :],
                                     AF.Copy, scale=1.0 / L)
            nc.vector.tensor_mul(r_wr[:, :], r_mean[:, :], r_mean[:, :])
            nc.vector.tensor_sub(r_msq[:, :], r_msq[:, :], r_wr[:, :])
            nc.scalar.activation(r_msq[:, :], r_msq[:, :], AF.Ln,
                                 bias=c.epscol[:1, :])
            nc.scalar.activation(r_std[:, :], r_msq[:, :], AF.Exp, scale=0.5)
            nc.scalar.activation(r_wr[:, :], r_msq[:, :], AF.Exp, scale=-0.5)
            rvw = fp.tile([1, N], F32, name="rvwrow", tag="rvwrow")
            nc.sync.dma_start(rvw[:, :], I["rvw_row"][:, :])
            nc.vector.tensor_mul(r_wr[:, :], r_wr[:, :], rvw[:, :])
            # sc = std / (rvw + 1e-10)   (for final denorm)
            t1 = fp.tile([1, N], F32, name="sct1", tag="sct1")
            nc.vector.tensor_scalar_add(t1[:, :], rvw[:, :], 1e-10)
            nc.vector.reciprocal(t1[:, :], t1[:, :])
            nc.vector.tensor_mul(r_sc[:, :], t1[:, :], r_std[:, :])

            mb = _bcast(c, fp, r_mean[:, :], 128, "rvm")
            wb = _bcast(c, fp, r_wr[:, :], 128, "rvw")
            bb = _bcast(c, fp, I["rvb_row"], 128, "rvb", via_dram=False)
            c.xn = []
            for ci in range(4):
                o = gp.tile([128, N], F32R, name=f"xn{ci}", tag=f"xn{ci}")
                d1 = fp.tile([128, N], F32, name="rvd", tag="rvd", bufs=2)
                nc.vector.tensor_sub(d1[:, :], X[ci][:, :].bitcast(F32), mb[:, :])
                nc.vector.tensor_mul(d1[:, :], d1[:, :], wb[:, :])
                nc.vector.tensor_add(o[:, :], d1[:, :], bb[:, :])
                c.xn.append(o)
            _dbg(c, "xn", [t[:, :] for t in c.xn])

            SE = _load_tiles(c, fp, "seaop_T")
            xsea = []
            for mc in range(4):
                t_ = fp.tile([128, N], F32R, name=f"xsea{mc}", tag=f"xsea{mc}")
                xsea.append(t_)
                for n0, nl in NC2:
                    ps = pm.tile([128, nl], F32, name="semm", tag="mm")
                    _matsum(c, ps, [SE[k][mc] for k in range(4)], c.xn, n0, nl)
                    nc.scalar.copy(t_[:, n0:n0 + nl], ps[:, :])
            EL = _load_tiles(c, fp, "emb_lhsT")
            emb_b = _load(c, fp, "emb_b")
            xt = []
            for mc in range(2):
                t_ = gp.tile([128, N], F32R, name=f"xtA{mc}", tag=f"xtA{mc}")
                xt.append(t_)
                for n0, nl in NC2:
                    ps = pm.tile([128, nl], F32, name="embmm", tag="mm")
                    _matsum(c, ps, [EL[k][mc] for k in range(4)], xsea, n0, nl)
                    nc.scalar.activation(t_[:, n0:n0 + nl], ps[:, :],
                                         AF.Identity,
                                         bias=emb_b[:, mc:mc + 1])
            _dbg(c, "x0", [t[:, :] for t in xt])

        # ======================================================== encoder
        for l in range(NLAYERS):
            with contextlib.ExitStack() as lst:
                lp = lst.enter_context(tc.tile_pool(name=f"lay{l}", bufs=1))
                rp = lst.enter_context(tc.tile_pool(name=f"rot{l}", bufs=2))
                pa = lst.enter_context(
                    tc.tile_pool(name=f"pda{l}", bufs=2, space="PSUM"))
                xt = _mamba_layer(c, l, lp, rp, pa, xt)
                if l == 0:
                    _dbg(c, "xl0", [t[:, :] for t in xt])

        # ======================================================== tail
        with contextlib.ExitStack() as tst:
            tp = tst.enter_context(tc.tile_pool(name="tail", bufs=1))
            encw = _load(c, tp, "encnw")
            encb = _load(c, tp, "encnb")
            xf = _layer_norm(c, tp, xt, encw, encb, c.gp, "xtB")
            PRJ = _load_tiles(c, tp, "proj_lhsT")
            projb = _load(c, tp, "projb")
            seaT = tp.tile([H, N], F32, name="seaT", tag="seaT")
            for n0, nl in NC2:
                ps = pm.tile([H, nl], F32, name="prmm", tag="mm")
                _matsum(c, ps, [PRJ[k][0] for k in range(2)], xf, n0, nl)
                nc.scalar.activation(seaT[:, n0:n0 + nl], ps[:, :], AF.Identity,
                                     bias=projb[:, :])
            _dbg(c, "sea", [seaT[:, :]])

            # trend extraction
            trt = []
            for s, ls in enumerate([512, 256, 128, 64]):
              with c.tc.tile_pool(name=f"wtr{s}", bufs=1) as wtr:
                TR = _load_tiles(c, wtr, f"trop{s}_T")
                mt = []
                for mc in range((ls + 127) // 128):
                    parts = min(128, ls - mc * 128)
                    t_ = tp.tile([parts, N], F32R, name=f"tr{s}_{mc}",
                                 tag=f"tr{s}_{mc}")
                    mt.append(t_)
                    for n0, nl in NC2:
                        ps = pm.tile([parts, nl], F32, name="trmm", tag="mm")
                        _matsum(c, ps, [TR[k][mc] for k in range(4)], c.xn,
                                n0, nl)
                        nc.scalar.copy(t_[:, n0:n0 + nl], ps[:, :])
                trt.append(mt)
            tr0, tr1, tr2, tr3 = trt

            def mixstep(low, i, high, hi_s):
              with c.tc.tile_pool(name=f"wu{i}", bufs=1) as wu:
                W1 = _load_tiles(c, wu, f"u{i}w1_lhsT")
                b1 = _load(c, wu, f"u{i}b1")
                W2 = _load_tiles(c, wu, f"u{i}w2_lhsT")
                b2 = _load(c, wu, f"u{i}b2")
                gt = []
                for mc in range(len(W1[0])):
                    parts = W1[0][mc].shape[1]
                    g_ = tp.tile([parts, N], F32R, name=f"mxg{i}_{mc}",
                                 tag=f"gA{mc}")
                    gt.append(g_)
                    for n0, nl in NC2:
                        ps = pm.tile([parts, nl], F32, name="mxmm", tag="mm")
                        _matsum(c, ps, [W1[k][mc] for k in range(len(W1))],
                                low, n0, nl)
                        nc.scalar.activation(
                            g_[:, n0:n0 + nl], ps[:, :], AF.Gelu,
                            bias=b1[:parts, mc:mc + 1])
                out = []
                for mc in range(len(W2[0])):
                    parts = W2[0][mc].shape[1]
                    o_ = high[mc]  # accumulate in place into the trend tile
                    out.append(o_)
                    for n0, nl in NC2:
                        ps = pm.tile([parts, nl], F32, name="mxmm2", tag="mm")
                        _matsum(c, ps, [W2[k][mc] for k in range(len(W2))],
                                gt, n0, nl)
                        b_ = tp.tile([parts, N], F32, name="mxb", tag="mxb",
                                     bufs=2)
                        nc.scalar.activation(
                            b_[:, n0:n0 + nl], ps[:, :], AF.Identity,
                            bias=b2[:parts, mc:mc + 1])
                        nc.vector.tensor_add(
                            o_[:, n0:n0 + nl],
                            o_[:, n0:n0 + nl].bitcast(F32),
                            b_[:, n0:n0 + nl])
                return out

            o1 = mixstep(tr3, 0, tr2, 2)
            o2 = mixstep(o1, 1, tr1, 1)
            o3 = mixstep(o2, 2, tr0, 0)

            MP = [_load_tiles(c, tp, f"map{s}_lhsT") for s in range(4)]
            mapb = _load(c, tp, "mapb")
            outst = [o3, o2, o1, tr3]
            treT = tp.tile([H, N], F32, name="treT", tag="treT")
            for n0, nl in NC2:
                ps = pm.tile([H, nl], F32, name="mpmm", tag="mm")
                ops = []
                for s in range(4):
                    for k in range(len(MP[s])):
                        ops.append((MP[s][k][0], outst[s][k]))
                for i, (w_, x_) in enumerate(ops):
                    nc.tensor.matmul(ps[:, :], w_[:, :], x_[:, n0:n0 + nl],
                                     start=(i == 0), stop=(i == len(ops) - 1))
                nc.scalar.activation(treT[:, n0:n0 + nl], ps[:, :], AF.Identity,
                                     bias=mapb[:, :])
            _dbg(c, "tre", [treT[:, :]])

            # final combine + RevIN denorm
            p1 = tp.tile([H, N], F32, name="fin1", tag="fin1")
            twb = _bcast(c, tp, I["trw_row"], H, "finb", via_dram=False)
            nc.vector.tensor_mul(p1[:, :], treT[:, :], twb[:, :])
            nc.vector.tensor_add(p1[:, :], p1[:, :], seaT[:, :])
            rbb = _bcast(c, tp, I["rvb_row"], H, "finb", via_dram=False)
            nc.vector.tensor_sub(p1[:, :], p1[:, :], rbb[:, :])
            scb = _bcast(c, tp, c.r_sc[:, :], H, "finb")
            nc.vector.tensor_mul(p1[:, :], p1[:, :], scb[:, :])
            mnb = _bcast(c, tp, c.r_mean[:, :], H, "finb")
            nc.vector.tensor_add(p1[:, :], p1[:, :], mnb[:, :])
            nc.sync.dma_start(c.out_pred[:, :], p1[:, :])


def _mamba_layer(c, l, lp, rp, pa, xt):
    nc, pm = c.nc, c.pm
    tc = c.tc
    import contextlib

    # scratch tags: scrA{g} sized [128,2N] bf16-or-[128,N] f32 (6896B),
    # scrB{g} [128,N] f32 (3448B)
    def scrA(g, shape, dtype, nm):
        return rp.tile(shape, dtype, name=nm, tag=f"scrA{g}", bufs=1)

    def scrB(g, shape, dtype, nm):
        return rp.tile(shape, dtype, name=nm, tag=f"scrB{g}", bufs=1)

    zt, xcs = [], []
    with c.tc.tile_pool(name=f"w1_{l}", bufs=1) as wp1:
        IL = _load_tiles(c, wp1, f"in_lhsT_{l}")
        cw0 = _load(c, lp, f"cw0_{l}")
        cw1 = _load(c, lp, f"cw1_{l}")
        cb = _load(c, lp, f"cb_{l}")
        xcraw = []
        for f in range(8):
            if f < 4:
                dst = scrA(f, [128, N], F32, f"xcraw{f}")
                xcraw.append(dst)
            else:
                dst = lp.tile([128, N], BF16, name=f"zt{f - 4}", tag=f"zt{f - 4}")
                zt.append(dst)
            for n0, nl in NC2:
                ps = pm.tile([128, nl], F32, name="inmm", tag="mm")
                _matsum(c, ps, [IL[k][f] for k in range(2)], xt, n0, nl)
                if f % 2 == 0:
                    nc.scalar.copy(dst[:, n0:n0 + nl], ps[:, :])
                else:
                    nc.vector.tensor_copy(dst[:, n0:n0 + nl], ps[:, :])
        # conv + silu -> xcs (f32r)
        for g in range(4):
            xcc = scrB(g, [128, N], F32, f"xcc{g}")
            nc.vector.tensor_scalar(xcc[:, :], xcraw[g][:, :], cw1[:, g:g + 1],
                                    cb[:, g:g + 1], AL.mult, AL.add)
            nc.vector.scalar_tensor_tensor(xcc[:, 1:], xcraw[g][:, :N - 1],
                                           cw0[:, g:g + 1], xcc[:, 1:],
                                           AL.mult, AL.add)
            e = scrA(g, [128, N], F32, f"cve{g}")
            nc.scalar.activation(e[:, :], xcc[:, :], AF.Exp, scale=-1.0)
            nc.vector.tensor_scalar_add(e[:, :], e[:, :], 1.0)
            nc.vector.reciprocal(e[:, :], e[:, :])
            o = lp.tile([128, N], F32R, name=f"xcs{g}", tag=f"xcs{g}")
            nc.vector.tensor_mul(o[:, :], xcc[:, :], e[:, :])
            xcs.append(o)

    # x_proj + dt
    dtT = []
    with c.tc.tile_pool(name=f"w2_{l}", bufs=1) as wp2:
        XPB = _load_tiles(c, wp2, f"xpbc_lhsT_{l}")
        XPD = _load_tiles(c, wp2, f"xpdt_lhsT_{l}")
        dtin = lp.tile([16, N], F32R, name="dtin", tag="dtin")
        bcrows = lp.tile([32, N], BF16, name="bcrows", tag="bcrows")
        for n0, nl in NC2:
            ps = pm.tile([32, nl], F32, name="xpmm", tag="mm")
            _matsum(c, ps, [XPB[k][0] for k in range(4)], xcs, n0, nl)
            nc.scalar.copy(bcrows[:, n0:n0 + nl], ps[:, :])
            ps2 = pm.tile([16, nl], F32, name="xpmm2", tag="mm")
            _matsum(c, ps2, [XPD[k][0] for k in range(4)], xcs, n0, nl)
            nc.scalar.copy(dtin[:, n0:n0 + nl], ps2[:, :])
        bc_dram = c.dp.tile([32, N], BF16, name=f"bcd{l}", tag="bc_dram")
        nc.sync.dma_start(bc_dram[:, :], bcrows[:, :])
        DTW = _load_tiles(c, wp2, f"dt_lhsT_{l}")
        dtb = _load(c, lp, f"dtb_{l}")
        for g in range(4):
            u = scrA(g, [128, N], F32, f"dtu{g}")
            for n0, nl in NC2:
                ps = pm.tile([128, nl], F32, name="dtmm", tag="mm")
                nc.tensor.matmul(ps[:, :], DTW[0][g][:, :], dtin[:, n0:n0 + nl],
                                 start=True, stop=True)
                nc.scalar.activation(u[:, n0:n0 + nl], ps[:, :], AF.Exp,
                                     bias=dtb[:, g:g + 1])
            dt_ = lp.tile([128, N], F32, name=f"dtT{g}", tag=f"dtT{g}")
            nc.scalar.activation(dt_[:, :], u[:, :], AF.Ln, bias=1.0)
            dtT.append(dt_)
    wT = []
    for g in range(4):
        w_ = lp.tile([128, N], BF16, name=f"wT{g}", tag=f"wT{g}")
        nc.vector.tensor_mul(w_[:, :], dtT[g][:, :], xcs[g][:, :].bitcast(F32))
        wT.append(w_)

    # ---- scan: 16 states s, grouped in pairs for the reduction tree
    ytile = [None] * 4
    for grp in range(8):
        tmp2 = [scrA(g, [128, 2 * N], BF16, f"tmp2_{g}") for g in range(4)]
        for si in range(2):
            s = grp * 2 + si
            Bb = rp.tile([128, N], BF16, name="Bb", tag="Bb", bufs=2)
            nc.sync.dma_start(Bb[:, :],
                                bc_dram[s:s + 1, :].broadcast_to([128, N]))
            Cb = rp.tile([128, N], BF16, name="Cb", tag="Cb", bufs=2)
            nc.sync.dma_start(Cb[:, :],
                                bc_dram[16 + s:17 + s, :].broadcast_to([128, N]))
            for g in range(4):
                da = pa.tile([128, N], F32, name="dA", tag="dA")
                nc.scalar.activation(da[:, :], dtT[g][:, :], AF.Exp,
                                     scale=float(-(s + 1)))
                dbx = rp.tile([128, N], BF16, name="dbx", tag="dbx", bufs=2)
                nc.vector.tensor_mul(dbx[:, :], wT[g][:, :], Bb[:, :])
                h = rp.tile([128, N], BF16, name="h", tag="h", bufs=2)
                nc.vector.tensor_tensor_scan(h[:, :], da[:, :], dbx[:, :], 0.0,
                                             AL.mult, AL.add)
                nc.vector.tensor_mul(tmp2[g][:, si * N:(si + 1) * N],
                                     h[:, :], Cb[:, :])
        for g in range(4):
            if grp == 0:
                y_ = scrB(g, [128, N], F32, f"y{g}")
                nc.vector.tensor_add(y_[:, :], tmp2[g][:, 0:N],
                                     tmp2[g][:, N:2 * N])
                ytile[g] = y_
            else:
                t01 = rp.tile([128, N], BF16, name="t01", tag="t01", bufs=2)
                nc.vector.tensor_add(t01[:, :], tmp2[g][:, 0:N],
                                     tmp2[g][:, N:2 * N])
                nc.vector.tensor_add(ytile[g][:, :], ytile[g][:, :], t01[:, :])

    # ---- gating
    Dcol = _load(c, lp, f"D_{l}")
    ym = []
    for g in range(4):
        yg = scrA(g, [128, N], F32, f"yg{g}")
        nc.vector.scalar_tensor_tensor(yg[:, :], xcs[g][:, :].bitcast(F32),
                                       Dcol[:, g:g + 1], ytile[g][:, :],
                                       AL.mult, AL.add)
        e2b = lp.tile([128, N], F32, name=f"gze{g}", tag=f"dtT{g}")
        nc.scalar.activation(e2b[:, :], zt[g][:, :], AF.Exp, scale=-1.0)
        nc.vector.tensor_scalar_add(e2b[:, :], e2b[:, :], 1.0)
        nc.vector.reciprocal(e2b[:, :], e2b[:, :])
        zr = scrB(g, [128, N], F32, f"zr{g}")
        nc.vector.tensor_mul(zr[:, :], zt[g][:, :], e2b[:, :])
        o = lp.tile([128, N], F32R, name=f"ym{g}", tag=f"xcs{g}")
        nc.vector.tensor_mul(o[:, :], yg[:, :], zr[:, :])
        ym.append(o)

    # ---- out_proj + exchange + LN1 + FFN + LN2
    with c.tc.tile_pool(name=f"w3_{l}", bufs=1) as wp3:
        OL = _load_tiles(c, wp3, f"out_lhsT_{l}")
        fT = []
        for mi in range(2):
            t_ = lp.tile([128, N], F32, name=f"fT{mi}", tag=f"fT{mi}")
            fT.append(t_)
            for n0, nl in NC2:
                ps = pm.tile([128, nl], F32, name="opmm", tag="mm")
                _matsum(c, ps, [OL[k][mi] for k in range(4)], ym, n0, nl)
                nc.scalar.copy(t_[:, n0:n0 + nl], ps[:, :])
        if l == 0:
            _dbg(c, "f0", [t[:, :] for t in fT])

        fdram = c.dp.tile([256, N], F32, name=f"fd{l}", tag="fdram")
        sdram = c.dp.tile([256, N], F32, name=f"sd{l}", tag="sdram")
        for mi in range(2):
            nc.sync.dma_start(fdram[mi * 128:(mi + 1) * 128, :], fT[mi][:, :])
        nc.gpsimd.collective_compute("AllReduce", AL.add, replica_groups=PAIRS,
                                     ins=[fdram.opt()], outs=[sdram.opt()])
        xnew = []
        for mi in range(2):
            s_ = scrA(mi, [128, N], F32, f"exs{mi}")
            nc.sync.dma_start(s_[:, :], sdram[mi * 128:(mi + 1) * 128, :])
            nc.vector.tensor_sub(s_[:, :], s_[:, :], fT[mi][:, :])
            dr = scrA(mi + 2, [128, N], F32, f"exd{mi}")
            nc.scalar.copy(dr[:, :], s_[:, ::-1])
            a1 = scrB(mi, [128, N], F32, f"exa{mi}")
            nc.vector.tensor_add(a1[:, :], xt[mi][:, :].bitcast(F32),
                                 fT[mi][:, :])
            xv = lp.tile([128, N], F32R, name=f"xnew{mi}", tag=f"wT{mi}")
            nc.vector.tensor_add(xv[:, :], a1[:, :], dr[:, :])
            xnew.append(xv)
        n1w = _load(c, lp, f"n1w_{l}")
        n1b = _load(c, lp, f"n1b_{l}")
        xln = _layer_norm(c, rp, xnew, n1w, n1b, lp, f"xln{l}_")

        F1 = _load_tiles(c, wp3, f"f1_lhsT_{l}")
        F2 = _load_tiles(c, wp3, f"f2_lhsT_{l}")
        f1b = _load(c, lp, f"f1b_{l}")
        f2b = _load(c, lp, f"f2b_{l}")
        h1 = []
        for mf in range(2):
            t_ = lp.tile([128, N], F32R, name=f"ffh{mf}", tag=f"xcs{mf}")
            h1.append(t_)
            for n0, nl in NC2:
                ps = pm.tile([128, nl], F32, name="f1mm", tag="mm")
                _matsum(c, ps, [F1[k][mf] for k in range(2)], xln, n0, nl)
                nc.scalar.activation(t_[:, n0:n0 + nl], ps[:, :],
                                     AF.Gelu,
                                     bias=f1b[:, mf:mf + 1])
        xe2 = []
        for mi in range(2):
            y2 = scrA(mi, [128, N], F32, f"ffy{mi}")
            for n0, nl in NC2:
                ps = pm.tile([128, nl], F32, name="f2mm", tag="mm")
                _matsum(c, ps, [F2[k][mi] for k in range(2)], h1, n0, nl)
                nc.scalar.activation(y2[:, n0:n0 + nl], ps[:, :], AF.Identity,
                                     bias=f2b[:, mi:mi + 1])
            xv = lp.tile([128, N], F32R, name=f"xe2{mi}", tag=f"xcs{mi + 2}")
            nc.vector.tensor_add(xv[:, :],
                                 xln[mi][:, :].bitcast(F32), y2[:, :])
            xe2.append(xv)
        n2w = _load(c, lp, f"n2w_{l}")
        n2b = _load(c, lp, f"n2b_{l}")
        xout = _layer_norm(c, rp, xe2, n2w, n2b, c.gp,
                           "xtB" if l % 2 == 0 else "xtA")
    return xout


# ---------------------------------------------------------------- entry
def _get_program():
    if "prog" not in _CACHE:
        _CACHE["prog"] = _build()
    return _CACHE["prog"]


def kernel(**inputs):
    nc = _get_program()
    in_maps = [make_core_inputs(inputs, c) for c in range(8)]
    res = run_bass_kernel_spmd(nc, in_maps, list(range(8))).results
    out = np.empty((B, H, N, 1), np.float32)
    for b in range(B):
        out[b, :, :, 0] = res[2 * b]["pred"]
    return out


if __name__ == "__main__":
    print("building program...")
    _get_program()
    print("built ok")



# revision 2
# speedup vs baseline: 8.8721x; 8.8721x over previous
"""DSTMamba Trainium2 kernel: 8 NeuronCores, SPMD, wire-optimized.

Core c handles (batch b=c//2, direction d=c%2). The axon tunnel to the
devices is a shared ~45MB/s pipe, so per-dispatch wire bytes dominate:
every unique byte is shipped exactly once. All weights + the 4 input
batches are packed into bf16 "group" matrices (grouped by column
count), each core uploads a 1/8 row-shard, and an on-device AllGather
reconstructs the full matrices in HBM on every core. Per-core
batch/direction specialization happens on device with mask-multiplies
(SPMD-safe): x = (sum_b x_b*m_b) merged with its time-reversal by
even/odd masks; direction-dependent Mamba weights are mask-merged from
both direction variants. Constant seasonal/trend operators are baked
into the NEFF (inline Const tensors, zero wire cost). Tiny
precision-sensitive vectors (RevIN rows, conv/dt/D columns) ride in a
per-core f32 sideband. The XLA executable is compiled once and cached;
outputs are bf16.

Device layouts are transposed: activations are [feature, time] tiles so
every matmul takes pre-transposed lhsT weights (bf16 converted to
float32r on device) and the Mamba recurrence is tensor_tensor_scan
along the free/time axis. The bidirectional merge is a pair AllReduce +
subtract-own-contribution + reversed copy (symmetric SPMD).
"""

import numpy as np
import ml_dtypes

import concourse.bacc as bacc
import concourse.mybir as mybir
from concourse import tile

B, L, H, N = 4, 512, 96, 862
DM, DS = 256, 16
DI = 512
DTR = 16
DFF, NLAYERS = 256, 2
DSL, KSTD = 3, 25
EPS = 1e-5

F32 = mybir.dt.float32
F32R = mybir.dt.float32r
BF16 = mybir.dt.bfloat16
AL = mybir.AluOpType
AF = mybir.ActivationFunctionType

NC2 = [(0, 512), (512, 350)]  # even moving-dim chunks covering N=862
PAIRS = [[0, 1], [2, 3], [4, 5], [6, 7]]
ALL8 = [[0, 1, 2, 3, 4, 5, 6, 7]]

_CACHE = {}

# ------------------------------------------------------------ wire layout
# Gathered bf16 groups: name -> cols; tensors -> (group, row_off, rows).
_GCOLS = {"gx": N, "g1024": 1024, "g512": 512, "g256": 256,
          "g128": 128, "g96": 96, "g48": 48, "gb": 46}


def _mk_glayout():
    lay, size = {}, {g: 0 for g in _GCOLS}

    def add(grp, key, rows):
        lay[key] = (grp, size[grp], rows)
        size[grp] += rows

    add("gx", "x", 4 * L)
    for l in range(NLAYERS):
        for d in range(2):
            add("g1024", f"in_{l}{d}", DM)
    for l in range(NLAYERS):
        for d in range(2):
            add("g512", f"dt_{l}{d}", DTR)
    add("g512", "u2w1", 256)
    add("g512", "u2w2", 512)
    for l in range(NLAYERS):
        for d in range(2):
            add("g256", f"out_{l}{d}", DI)
    add("g256", "emb", L)
    for l in range(NLAYERS):
        add("g256", f"f1_{l}", DM)
        add("g256", f"f2_{l}", DFF)
    add("g256", "u1w1", 128)
    add("g256", "u1w2", 256)
    add("g128", "u0w1", 64)
    add("g128", "u0w2", 128)
    add("g96", "proj", DM)
    for s, ls in enumerate([512, 256, 128, 64]):
        add("g96", f"map{s}", ls)
    for l in range(NLAYERS):
        for d in range(2):
            add("g48", f"xp_{l}{d}", DI)
    add("gb", "biases", 128)
    for g, sz in size.items():
        assert sz % 8 == 0, (g, sz)
    return lay, size


_GLAY, _GSIZE = _mk_glayout()

# gb column layout: key -> (col_off, cols)
def _mk_bcols():
    bc, off = {}, 0

    def add(key, k):
        nonlocal off
        bc[key] = (off, k)
        off += k

    add("emb_b", 2)
    for l in range(NLAYERS):
        for k in ["n1w", "n1b", "n2w", "n2b", "f1b", "f2b"]:
            add(f"{k}_{l}", 2)
    add("encnw", 2)
    add("encnb", 2)
    add("projb", 1)
    add("mapb", 1)
    add("u0b1", 1)
    add("u0b2", 1)
    add("u1b1", 2)
    add("u1b2", 2)
    add("u2b1", 4)
    add("u2b2", 4)
    assert off == _GCOLS["gb"], off
    return bc


_BCOLS = _mk_bcols()

# priv f32 [1, 2592]: rvw(862) rvb(862) trw(862) me mo mb0..mb3
PRIV_RVW, PRIV_RVB, PRIV_TRW = 0, N, 2 * N
PRIV_MASK = 3 * N
PRIV_LEN = 3 * N + 6
# privcol f32 [128, 40]: per layer l, per j in [cw0,cw1,cb,dtb,D]: 4 cols
PCOL_KEYS = ["cw0", "cw1", "cb", "dtb", "D"]


# ---------------------------------------------------------------- host math
def _mavg_matrix(length):
    M = np.zeros((length, length), np.float64)
    p = (KSTD - 1) // 2
    for i in range(length):
        for d in range(-p, p + 1):
            j = min(max(i + d, 0), length - 1)
            M[i, j] += 1.0 / KSTD
    return M


def _pool_matrix(lo, hi):
    P = np.zeros((lo, hi), np.float64)
    for i in range(lo):
        P[i, 2 * i] = 0.5
        P[i, 2 * i + 1] = 0.5
    return P


def _trend_ops():
    ops = []
    P = np.eye(L)
    cur = L
    for s in range(DSL + 1):
        ops.append(_mavg_matrix(cur) @ P)
        if s < DSL:
            P = _pool_matrix(cur // 2, cur) @ P
            cur //= 2
    return ops  # [512,512],[256,512],[128,512],[64,512]


def _col(v):
    v = np.asarray(v, np.float32).reshape(-1)
    if v.size <= 128:
        out = np.zeros((128, 1), np.float32)
        out[:v.size, 0] = v
        return out
    return np.ascontiguousarray(v.reshape(-1, 128).T)


def _t(m):
    return np.ascontiguousarray(np.asarray(m, np.float32).T)


def make_core_inputs_all(inputs):
    """Pack full inputs into 8 per-core maps (group shards + priv)."""
    g = lambda k: np.asarray(inputs[k], np.float32)
    bf = ml_dtypes.bfloat16

    # ---- build group matrices (shared content, shipped sharded)
    gm = {name: np.zeros((rows, _GCOLS[name]), np.float32)
          for name, rows in _GSIZE.items()}

    def put(key, mat):
        grp, off, rows = _GLAY[key]
        assert mat.shape == (rows, _GCOLS[grp]), (key, mat.shape)
        gm[grp][off:off + rows] = mat

    x = g("history_data")[:, :, :, 0]          # [B,L,N]
    put("x", x.reshape(B * L, N))
    for l in range(NLAYERS):
        for d in range(2):
            put(f"in_{l}{d}", _t(g("m_in")[l, d]))
            put(f"dt_{l}{d}", _t(g("m_dt_w")[l, d]))
            put(f"out_{l}{d}", _t(g("m_out")[l, d]))
            put(f"xp_{l}{d}", _t(g("m_xproj")[l, d]))
    put("emb", _t(g("emb_w")))
    for l in range(NLAYERS):
        put(f"f1_{l}", _t(g("f1_w")[l]))
        put(f"f2_{l}", _t(g("f2_w")[l]))
    put("u0w1", _t(g("u0w1")))
    put("u0w2", _t(g("u0w2")))
    put("u1w1", _t(g("u1w1")))
    put("u1w2", _t(g("u1w2")))
    put("u2w1", _t(g("u2w1")))
    put("u2w2", _t(g("u2w2")))
    put("proj", _t(g("proj_w")))
    for s in range(4):
        put(f"map{s}", _t(g(f"map{s}_w")))

    bias = np.zeros((128, _GCOLS["gb"]), np.float32)

    def putb(key, v):
        off, k = _BCOLS[key]
        bias[:, off:off + k] = _col(v)[:, :k] if v.size > 128 else _col(v)

    putb("emb_b", g("emb_b"))
    for l in range(NLAYERS):
        putb(f"n1w_{l}", g("n1_w")[l])
        putb(f"n1b_{l}", g("n1_b")[l])
        putb(f"n2w_{l}", g("n2_w")[l])
        putb(f"n2b_{l}", g("n2_b")[l])
        putb(f"f1b_{l}", g("f1_b")[l])
        putb(f"f2b_{l}", g("f2_b")[l])
    putb("encnw", g("encn_w"))
    putb("encnb", g("encn_b"))
    putb("projb", g("proj_b"))
    putb("mapb", sum(g(f"map{s}_b") for s in range(4)))
    for i in range(3):
        putb(f"u{i}b1", g(f"u{i}b1"))
        putb(f"u{i}b2", g(f"u{i}b2"))
    grp, off, rows = _GLAY["biases"]
    gm[grp][off:off + rows] = bias

    gmb = {name: np.ascontiguousarray(m.astype(bf)) for name, m in gm.items()}

    # ---- per-core maps
    maps = []
    for c in range(8):
        b, d = c // 2, c % 2
        m = {}
        for name, mat in gmb.items():
            rs = mat.shape[0] // 8
            m[f"sh_{name}"] = np.ascontiguousarray(mat[c * rs:(c + 1) * rs])
        priv = np.zeros((1, PRIV_LEN), np.float32)
        rvw, rvb, trw = g("revin_w"), g("revin_b"), g("tre_w")
        if d == 1:
            rvw, rvb, trw = rvw[::-1], rvb[::-1], trw[::-1]
        priv[0, PRIV_RVW:PRIV_RVW + N] = rvw
        priv[0, PRIV_RVB:PRIV_RVB + N] = rvb
        priv[0, PRIV_TRW:PRIV_TRW + N] = trw
        priv[0, PRIV_MASK + 0] = 1.0 if d == 0 else 0.0
        priv[0, PRIV_MASK + 1] = 1.0 if d == 1 else 0.0
        for bb in range(4):
            priv[0, PRIV_MASK + 2 + bb] = 1.0 if bb == b else 0.0
        m["priv"] = priv
        pc = np.zeros((128, 5 * NLAYERS * 4), np.float32)
        for l in range(NLAYERS):
            vals = [g("m_conv_w")[l, d, :, 0], g("m_conv_w")[l, d, :, 1],
                    g("m_conv_b")[l, d], g("m_dt_b")[l, d], g("m_D")[l, d]]
            for j, v in enumerate(vals):
                pc[:, (l * 5 + j) * 4:(l * 5 + j) * 4 + 4] = _col(v)
        m["privcol"] = pc
        maps.append(m)
    return maps


# ------------------------------------------------------------- device build
class _Ctx:
    pass


def _build():
    nc = bacc.Bacc("TRN2", target_bir_lowering=False, debug=False,
                   num_devices=8)

    I = {}
    for name, rows in _GSIZE.items():
        I[f"sh_{name}"] = nc.dram_tensor(
            f"sh_{name}", [rows // 8, _GCOLS[name]], BF16,
            kind="ExternalInput").ap()
    I["priv"] = nc.dram_tensor("priv", [1, PRIV_LEN], F32,
                               kind="ExternalInput").ap()
    I["privcol"] = nc.dram_tensor("privcol", [128, 5 * NLAYERS * 4], F32,
                                  kind="ExternalInput").ap()

    # constants baked into the NEFF
    tops = _trend_ops()
    consts = {"seaop_T": _t(np.eye(L) - tops[0]),
              "ones_col": np.ones((128, 1), np.float32)}
    for s in range(4):
        consts[f"trop{s}_T"] = _t(tops[s])
    C = {k: nc.inline_tensor(v.astype(np.float32), name=k).ap()
         for k, v in consts.items()}

    out_pred = nc.dram_tensor("pred", [H, N], BF16, kind="ExternalOutput").ap()

    c = _Ctx()
    c.nc, c.I, c.C, c.out_pred = nc, I, C, out_pred
    with tile.TileContext(nc) as tc:
        c.tc = tc
        _emit(c)
    nc.compile()
    return nc


def _gap(c, key):
    """gathered AP region for a packed tensor key -> (ap, row_off, rows, cols)"""
    grp, off, rows = _GLAY[key]
    return c.gath[grp], off, rows, _GCOLS[grp]


def _load_tiles_g(c, pool, key, tag=None):
    """shared bf16 weight -> [128,128]-chunked F32R tiles (convert on load)."""
    nc = c.nc
    gap, r0, K, M = _gap(c, key)
    out = []
    for ko in range(0, K, 128):
        rowt = []
        for mo in range(0, M, 128):
            kk, mm = min(128, K - ko), min(128, M - mo)
            tb = pool.tile([kk, mm], BF16, name=f"{key}b_{ko}_{mo}",
                           tag="gldb", bufs=3)
            nc.sync.dma_start(tb[:, :], gap[r0 + ko:r0 + ko + kk, mo:mo + mm])
            t_ = pool.tile([kk, mm], F32R, name=f"{key}_{ko}_{mo}",
                           tag=f"{tag or key}_{ko}_{mo}")
            nc.scalar.copy(t_[:, :], tb[:, :])
            rowt.append(t_)
        out.append(rowt)
    return out


def _load_tiles_dir(c, pool, base, l, tag=None):
    """dir-dependent weight: mask-merge both dir variants -> F32R tiles."""
    nc = c.nc
    gap0, r00, K, M = _gap(c, f"{base}_{l}0")
    gap1, r01, _, _ = _gap(c, f"{base}_{l}1")
    out = []
    for ko in range(0, K, 128):
        rowt = []
        for mo in range(0, M, 128):
            kk, mm = min(128, K - ko), min(128, M - mo)
            t0 = pool.tile([kk, mm], BF16, name=f"{base}{l}a", tag="mrga",
                           bufs=3)
            nc.sync.dma_start(t0[:, :], gap0[r00 + ko:r00 + ko + kk,
                                             mo:mo + mm])
            t1 = pool.tile([kk, mm], BF16, name=f"{base}{l}b", tag="mrgb",
                           bufs=3)
            nc.sync.dma_start(t1[:, :], gap1[r01 + ko:r01 + ko + kk,
                                             mo:mo + mm])
            t_ = pool.tile([kk, mm], F32R, name=f"{base}{l}_{ko}_{mo}",
                           tag=f"{tag or base}_{ko}_{mo}")
            nc.vector.tensor_scalar(t_[:, :], t0[:, :], c.mdir0[:kk, :], None,
                                    AL.mult)
            nc.vector.scalar_tensor_tensor(t_[:, :], t1[:, :], c.mdir1[:kk, :],
                                           t_[:, :], AL.mult, AL.add)
            rowt.append(t_)
        out.append(rowt)
    return out


def _load_cols(c, pool, key):
    """bias pack columns -> F32 [128,k] tile."""
    nc = c.nc
    gap, r0, rows, _ = _gap(c, "biases")
    off, k = _BCOLS[key]
    tb = pool.tile([128, k], BF16, name=f"{key}b", tag="bldb", bufs=3)
    nc.sync.dma_start(tb[:, :], gap[r0:r0 + 128, off:off + k])
    t_ = pool.tile([128, k], F32, name=key, tag=key)
    nc.vector.tensor_copy(t_[:, :], tb[:, :])
    return t_


def _priv_cols(c, pool, l, j):
    """per-core f32 sideband column pack -> [128,4] F32 tile."""
    key = PCOL_KEYS[j]
    t_ = pool.tile([128, 4], F32, name=f"{key}_{l}", tag=f"{key}_{l}")
    base = (l * 5 + j) * 4
    c.nc.sync.dma_start(t_[:, :], c.I["privcol"][:, base:base + 4])
    return t_


def _bcast(c, pool, row_ap, parts, tag, via_dram=True):
    """broadcast [1,N] (sbuf or dram) row to [parts, N] f32 sbuf tile."""
    nc = c.nc
    if via_dram:
        d = c.dp.tile([1, N], F32, name=f"bd_{tag}", tag=f"bd_{tag}")
        nc.sync.dma_start(d[:, :], row_ap.bitcast(F32))
        src = d[:, :]
    else:
        src = row_ap.bitcast(F32)
    bt = pool.tile([parts, N], F32, name=f"bc_{tag}", tag=f"bc_{tag}")
    nc.sync.dma_start(bt[:, :], src.broadcast_to([parts, N]))
    return bt


def _matsum(c, psum, lhs_tiles, rhs_tiles, n0, nl):
    """psum += sum_k lhs_tiles[k].T @ rhs_tiles[k][:, n0:n0+nl]"""
    nc = c.nc
    kn = len(lhs_tiles)
    for k in range(kn):
        nc.tensor.matmul(psum[:, :], lhs_tiles[k][:, :],
                         rhs_tiles[k][:, n0:n0 + nl],
                         start=(k == 0), stop=(k == kn - 1))


def _layer_norm(c, scr, xin, wcol, bcol, outpool, outtag):
    """xin: 2 [128,N] f32r tiles -> 2 [128,N] f32r tiles (norm over 256)."""
    nc, pm = c.nc, c.pm
    mrow = scr.tile([1, N], F32, name=f"lnm_{outtag}", tag="ln_mrow")
    qrow = scr.tile([1, N], F32, name=f"lnq_{outtag}", tag="ln_qrow")
    for n0, nl in NC2:
        ps = pm.tile([1, nl], F32, name="lnps", tag="mm1")
        for mi in range(2):
            nc.tensor.matmul(ps[:, :], c.ones_col[:, :], xin[mi][:, n0:n0 + nl],
                             start=(mi == 0), stop=(mi == 1))
        nc.scalar.activation(mrow[:, n0:n0 + nl], ps[:, :], AF.Copy,
                             scale=1.0 / DM)
        ps2 = pm.tile([1, nl], F32, name="lnps2", tag="mm1")
        for mi in range(2):
            sq = scr.tile([128, N], F32R, name="lnsq", tag="sq", bufs=2)
            nc.scalar.activation(sq[:, n0:n0 + nl],
                                 xin[mi][:, n0:n0 + nl].bitcast(F32), AF.Square)
            nc.tensor.matmul(ps2[:, :], c.ones_col[:, :], sq[:, n0:n0 + nl],
                             start=(mi == 0), stop=(mi == 1))
        nc.scalar.activation(qrow[:, n0:n0 + nl], ps2[:, :], AF.Copy,
                             scale=1.0 / DM)
    tmp_ = scr.tile([1, N], F32, name=f"lnt_{outtag}", tag="ln_trow")
    nc.vector.tensor_mul(tmp_[:, :], mrow[:, :], mrow[:, :])
    nc.vector.tensor_sub(qrow[:, :], qrow[:, :], tmp_[:, :])
    nc.scalar.activation(qrow[:, :], qrow[:, :], AF.Ln, bias=c.epscol[:1, :])
    nc.scalar.activation(qrow[:, :], qrow[:, :], AF.Exp, scale=-0.5)
    mb = _bcast(c, scr, mrow[:, :], 128, "lnm")
    rb = _bcast(c, scr, qrow[:, :], 128, "lnr")
    out = []
    for mi in range(2):
        o = outpool.tile([128, N], F32R, name=f"{outtag}{mi}", tag=f"{outtag}{mi}")
        d1 = scr.tile([128, N], F32, name="lnd1", tag="d1", bufs=2)
        nc.vector.tensor_sub(d1[:, :], xin[mi][:, :].bitcast(F32), mb[:, :])
        nc.vector.tensor_mul(d1[:, :], d1[:, :], rb[:, :])
        nc.vector.tensor_scalar(o[:, :], d1[:, :],
                                wcol[:, mi:mi + 1],
                                bcol[:, mi:mi + 1], AL.mult, AL.add)
        out.append(o)
    return out


def _load_tiles_const(c, pool, key, tag=None):
    ap = c.C[key]
    K, M = ap.shape
    out = []
    for ko in range(0, K, 128):
        rowt = []
        for mo in range(0, M, 128):
            kk, mm = min(128, K - ko), min(128, M - mo)
            t_ = pool.tile([kk, mm], F32R, name=f"{key}_{ko}_{mo}",
                           tag=f"{tag or key}_{ko}_{mo}")
            c.nc.sync.dma_start(t_[:, :],
                                ap[ko:ko + kk, mo:mo + mm].bitcast(F32R))
            rowt.append(t_)
        out.append(rowt)
    return out


def _emit(c):
    nc, tc, I = c.nc, c.tc, c.I
    import contextlib
    with contextlib.ExitStack() as est:
        gp = est.enter_context(tc.tile_pool(name="glob", bufs=1))
        pm = est.enter_context(tc.tile_pool(name="pmm", bufs=2, space="PSUM"))
        dp = est.enter_context(tc.tile_pool(name="drm", bufs=1, space="DRAM"))
        c.gp, c.pm, c.dp = gp, pm, dp

        # ---- prologue: stage shards + AllGather groups into HBM
        c.gath = {}
        for name, rows in _GSIZE.items():
            cols = _GCOLS[name]
            stage = nc.dram_tensor(f"st_{name}", [rows // 8, cols], BF16,
                                   kind="Internal").ap()
            nc.sync.dma_start(stage[:, :], I[f"sh_{name}"][:, :])
            gath = nc.dram_tensor(f"ga_{name}", [rows, cols], BF16,
                                  kind="Internal", addr_space="Shared").ap()
            nc.gpsimd.collective_compute(
                "AllGather", AL.bypass, replica_groups=ALL8,
                ins=[stage], outs=[gath])
            c.gath[name] = gath

        # ---- masks from priv
        def mk_mask(i, nm):
            t_ = gp.tile([128, 1], F32, name=nm, tag=nm)
            nc.sync.dma_start(
                t_[:, :],
                I["priv"][0:1, PRIV_MASK + i:PRIV_MASK + i + 1]
                .broadcast_to([128, 1]))
            return t_

        c.mdir0 = mk_mask(0, "mdir0")
        c.mdir1 = mk_mask(1, "mdir1")
        mbat = [mk_mask(2 + bb, f"mbat{bb}") for bb in range(4)]

        c.ones_col = gp.tile([128, 1], F32R, name="ones_col", tag="ones_col")
        nc.sync.dma_start(c.ones_col[:, :], c.C["ones_col"][:, :].bitcast(F32R))
        epscol = gp.tile([128, 1], F32, name="epscol", tag="epscol")
        c.nc.gpsimd.memset(epscol[:, :], EPS)
        c.epscol = epscol
        r_mean = gp.tile([1, N], F32, name="r_mean", tag="r_mean")
        r_std = gp.tile([1, N], F32, name="r_std", tag="r_std")
        r_wr = gp.tile([1, N], F32, name="r_wr", tag="r_wr")
        r_sc = gp.tile([1, N], F32, name="r_sc", tag="r_sc")
        c.r_mean, c.r_sc = r_mean, r_sc

        # ======================================================== stage A+B
        with tc.tile_pool(name="front", bufs=1) as fp:
            r_msq = fp.tile([1, N], F32, name="r_msq", tag="r_msq")
            gx, xr0, _, _ = _gap(c, "x")
            X = []
            for ci in range(4):
                acc = fp.tile([128, N], F32, name=f"xacc{ci}", tag="xacc",
                              bufs=2)
                for bb in range(4):
                    xb = fp.tile([128, N], BF16, name="xbload", tag="xbload",
                                 bufs=3)
                    nc.sync.dma_start(
                        xb[:, :],
                        gx[xr0 + bb * L + ci * 128:
                           xr0 + bb * L + (ci + 1) * 128, :])
                    if bb == 0:
                        nc.vector.tensor_scalar(acc[:, :], xb[:, :],
                                                mbat[0][:, :], None, AL.mult)
                    else:
                        nc.vector.scalar_tensor_tensor(
                            acc[:, :], xb[:, :], mbat[bb][:, :], acc[:, :],
                            AL.mult, AL.add)
                xrv = fp.tile([128, N], F32, name="xrev", tag="xrev", bufs=2)
                nc.scalar.copy(xrv[:, :], acc[:, ::-1])
                t_ = fp.tile([128, N], F32R, name=f"xin{ci}", tag=f"xin{ci}")
                nc.vector.tensor_scalar(t_[:, :], acc[:, :], c.mdir0[:, :],
                                        None, AL.mult)
                nc.vector.scalar_tensor_tensor(t_[:, :], xrv[:, :],
                                               c.mdir1[:, :], t_[:, :],
                                               AL.mult, AL.add)
                X.append(t_)
            for n0, nl in NC2:
                ps = pm.tile([1, nl], F32, name="rvs", tag="mm1")
                for ci in range(4):
                    nc.tensor.matmul(ps[:, :], c.ones_col[:, :],
                                     X[ci][:, n0:n0 + nl],
                                     start=(ci == 0), stop=(ci == 3))
                nc.scalar.activation(r_mean[:, n0:n0 + nl], ps[:, :],
                                     AF.Copy, scale=1.0 / L)
                ps2 = pm.tile([1, nl], F32, name="rvq", tag="mm1")
                for ci in range(4):
                    sq = fp.tile([128, N], F32R, name="rvsq", tag="sq", bufs=2)
                    nc.scalar.activation(sq[:, n0:n0 + nl],
                                         X[ci][:, n0:n0 + nl].bitcast(F32),
                                         AF.Square)
                    nc.tensor.matmul(ps2[:, :], c.ones_col[:, :],
                                     sq[:, n0:n0 + nl],
                                     start=(ci == 0), stop=(ci == 3))
                nc.scalar.activation(r_msq[:, n0:n0 + nl], ps2[:, :],
                                     AF.Copy, scale=1.0 / L)
            nc.vector.tensor_mul(r_wr[:, :], r_mean[:, :], r_mean[:, :])
            nc.vector.tensor_sub(r_msq[:, :], r_msq[:, :], r_wr[:, :])
            nc.scalar.activation(r_msq[:, :], r_msq[:, :], AF.Ln,
                                 bias=c.epscol[:1, :])
            nc.scalar.activation(r_std[:, :], r_msq[:, :], AF.Exp, scale=0.5)
            nc.scalar.activation(r_wr[:, :], r_msq[:, :], AF.Exp, scale=-0.5)
            rvw = fp.tile([1, N], F32, name="rvwrow", tag="rvwrow")
            nc.sync.dma_start(rvw[:, :], I["priv"][0:1, PRIV_RVW:PRIV_RVW + N])
            nc.vector.tensor_mul(r_wr[:, :], r_wr[:, :], rvw[:, :])
            # sc = std / (rvw + 1e-10)   (for final denorm)
            t1 = fp.tile([1, N], F32, name="sct1", tag="sct1")
            nc.vector.tensor_scalar_add(t1[:, :], rvw[:, :], 1e-10)
            nc.vector.reciprocal(t1[:, :], t1[:, :])
            nc.vector.tensor_mul(r_sc[:, :], t1[:, :], r_std[:, :])

            mb = _bcast(c, fp, r_mean[:, :], 128, "rvm")
            wb = _bcast(c, fp, r_wr[:, :], 128, "rvw")
            bb = _bcast(c, fp, I["priv"][0:1, PRIV_RVB:PRIV_RVB + N], 128,
                        "rvb", via_dram=False)
            c.xn = []
            for ci in range(4):
                o = gp.tile([128, N], F32R, name=f"xn{ci}", tag=f"xn{ci}")
                d1 = fp.tile([128, N], F32, name="rvd", tag="rvd", bufs=2)
                nc.vector.tensor_sub(d1[:, :], X[ci][:, :].bitcast(F32), mb[:, :])
                nc.vector.tensor_mul(d1[:, :], d1[:, :], wb[:, :])
                nc.vector.tensor_add(o[:, :], d1[:, :], bb[:, :])
                c.xn.append(o)

            SE = _load_tiles_const(c, fp, "seaop_T")
            xsea = []
            for mc in range(4):
                t_ = fp.tile([128, N], F32R, name=f"xsea{mc}", tag=f"xsea{mc}")
                xsea.append(t_)
                for n0, nl in NC2:
                    ps = pm.tile([128, nl], F32, name="semm", tag="mm")
                    _matsum(c, ps, [SE[k][mc] for k in range(4)], c.xn, n0, nl)
                    nc.scalar.copy(t_[:, n0:n0 + nl], ps[:, :])
            EL = _load_tiles_g(c, fp, "emb")
            emb_b = _load_cols(c, fp, "emb_b")
            xt = []
            for mc in range(2):
                t_ = gp.tile([128, N], F32R, name=f"xtA{mc}", tag=f"xtA{mc}")
                xt.append(t_)
                for n0, nl in NC2:
                    ps = pm.tile([128, nl], F32, name="embmm", tag="mm")
                    _matsum(c, ps, [EL[k][mc] for k in range(4)], xsea, n0, nl)
                    nc.scalar.activation(t_[:, n0:n0 + nl], ps[:, :],
                                         AF.Identity,
                                         bias=emb_b[:, mc:mc + 1])

        # ======================================================== encoder
        for l in range(NLAYERS):
            with contextlib.ExitStack() as lst:
                lp = lst.enter_context(tc.tile_pool(name=f"lay{l}", bufs=1))
                rp = lst.enter_context(tc.tile_pool(name=f"rot{l}", bufs=2))
                pa = lst.enter_context(
                    tc.tile_pool(name=f"pda{l}", bufs=2, space="PSUM"))
                xt = _mamba_layer(c, l, lp, rp, pa, xt)

        # ======================================================== tail
        with contextlib.ExitStack() as tst:
            tp = tst.enter_context(tc.tile_pool(name="tail", bufs=1))
            encw = _load_cols(c, tp, "encnw")
            encb = _load_cols(c, tp, "encnb")
            xf = _layer_norm(c, tp, xt, encw, encb, c.gp, "xtB")
            PRJ = _load_tiles_g(c, tp, "proj")
            projb = _load_cols(c, tp, "projb")
            seaT = tp.tile([H, N], F32, name="seaT", tag="seaT")
            for n0, nl in NC2:
                ps = pm.tile([H, nl], F32, name="prmm", tag="mm")
                _matsum(c, ps, [PRJ[k][0] for k in range(2)], xf, n0, nl)
                nc.scalar.activation(seaT[:, n0:n0 + nl], ps[:, :], AF.Identity,
                                     bias=projb[:H, :])

            # trend extraction
            trt = []
            for s, ls in enumerate([512, 256, 128, 64]):
              with c.tc.tile_pool(name=f"wtr{s}", bufs=1) as wtr:
                TR = _load_tiles_const(c, wtr, f"trop{s}_T")
                mt = []
                for mc in range((ls + 127) // 128):
                    parts = min(128, ls - mc * 128)
                    t_ = tp.tile([parts, N], F32R, name=f"tr{s}_{mc}",
                                 tag=f"tr{s}_{mc}")
                    mt.append(t_)
                    for n0, nl in NC2:
                        ps = pm.tile([parts, nl], F32, name="trmm", tag="mm")
                        _matsum(c, ps, [TR[k][mc] for k in range(4)], c.xn,
                                n0, nl)
                        nc.scalar.copy(t_[:, n0:n0 + nl], ps[:, :])
                trt.append(mt)
            tr0, tr1, tr2, tr3 = trt

            def mixstep(low, i, high, hi_s):
              with c.tc.tile_pool(name=f"wu{i}", bufs=1) as wu:
                W1 = _load_tiles_g(c, wu, f"u{i}w1")
                b1 = _load_cols(c, tp, f"u{i}b1")
                W2 = _load_tiles_g(c, wu, f"u{i}w2")
                b2 = _load_cols(c, tp, f"u{i}b2")
                gt = []
                for mc in range(len(W1[0])):
                    parts = W1[0][mc].shape[1]
                    g_ = tp.tile([parts, N], F32R, name=f"mxg{i}_{mc}",
                                 tag=f"gA{mc}")
                    gt.append(g_)
                    for n0, nl in NC2:
                        ps = pm.tile([parts, nl], F32, name="mxmm", tag="mm")
                        _matsum(c, ps, [W1[k][mc] for k in range(len(W1))],
                                low, n0, nl)
                        nc.scalar.activation(
                            g_[:, n0:n0 + nl], ps[:, :], AF.Gelu,
                            bias=b1[:parts, mc:mc + 1])
                out = []
                for mc in range(len(W2[0])):
                    parts = W2[0][mc].shape[1]
                    o_ = high[mc]  # accumulate in place into the trend tile
                    out.append(o_)
                    for n0, nl in NC2:
                        ps = pm.tile([parts, nl], F32, name="mxmm2", tag="mm")
                        _matsum(c, ps, [W2[k][mc] for k in range(len(W2))],
                                gt, n0, nl)
                        b_ = tp.tile([parts, N], F32, name="mxb", tag="mxb",
                                     bufs=2)
                        nc.scalar.activation(
                            b_[:, n0:n0 + nl], ps[:, :], AF.Identity,
                            bias=b2[:parts, mc:mc + 1])
                        nc.vector.tensor_add(
                            o_[:, n0:n0 + nl],
                            o_[:, n0:n0 + nl].bitcast(F32),
                            b_[:, n0:n0 + nl])
                return out

            o1 = mixstep(tr3, 0, tr2, 2)
            o2 = mixstep(o1, 1, tr1, 1)
            o3 = mixstep(o2, 2, tr0, 0)

            MP = [_load_tiles_g(c, tp, f"map{s}") for s in range(4)]
            mapb = _load_cols(c, tp, "mapb")
            outst = [o3, o2, o1, tr3]
            treT = tp.tile([H, N], F32, name="treT", tag="treT")
            for n0, nl in NC2:
                ps = pm.tile([H, nl], F32, name="mpmm", tag="mm")
                ops = []
                for s in range(4):
                    for k in range(len(MP[s])):
                        ops.append((MP[s][k][0], outst[s][k]))
                for i, (w_, x_) in enumerate(ops):
                    nc.tensor.matmul(ps[:, :], w_[:, :], x_[:, n0:n0 + nl],
                                     start=(i == 0), stop=(i == len(ops) - 1))
                nc.scalar.activation(treT[:, n0:n0 + nl], ps[:, :], AF.Identity,
                                     bias=mapb[:H, :])

            # final combine + RevIN denorm
            p1 = tp.tile([H, N], F32, name="fin1", tag="fin1")
            twb = _bcast(c, tp, I["priv"][0:1, PRIV_TRW:PRIV_TRW + N], H,
                         "finb", via_dram=False)
            nc.vector.tensor_mul(p1[:, :], treT[:, :], twb[:, :])
            nc.vector.tensor_add(p1[:, :], p1[:, :], seaT[:, :])
            rbb = _bcast(c, tp, I["priv"][0:1, PRIV_RVB:PRIV_RVB + N], H,
                         "finb", via_dram=False)
            nc.vector.tensor_sub(p1[:, :], p1[:, :], rbb[:, :])
            scb = _bcast(c, tp, c.r_sc[:, :], H, "finb")
            nc.vector.tensor_mul(p1[:, :], p1[:, :], scb[:, :])
            mnb = _bcast(c, tp, c.r_mean[:, :], H, "finb")
            pb = tp.tile([H, N], BF16, name="predb", tag="predb")
            nc.vector.tensor_add(pb[:, :], p1[:, :], mnb[:, :])
            nc.sync.dma_start(c.out_pred[:, :], pb[:, :])


def _mamba_layer(c, l, lp, rp, pa, xt):
    nc, pm = c.nc, c.pm

    # scratch tags: scrA{g} sized [128,2N] bf16-or-[128,N] f32 (6896B),
    # scrB{g} [128,N] f32 (3448B)
    def scrA(g, shape, dtype, nm):
        return rp.tile(shape, dtype, name=nm, tag=f"scrA{g}", bufs=1)

    def scrB(g, shape, dtype, nm):
        return rp.tile(shape, dtype, name=nm, tag=f"scrB{g}", bufs=1)

    zt, xcs = [], []
    with c.tc.tile_pool(name=f"w1_{l}", bufs=1) as wp1:
        IL = _load_tiles_dir(c, wp1, "in", l, tag="inl")
        cw0 = _priv_cols(c, lp, l, 0)
        cw1 = _priv_cols(c, lp, l, 1)
        cb = _priv_cols(c, lp, l, 2)
        xcraw = []
        for f in range(8):
            if f < 4:
                dst = scrA(f, [128, N], F32, f"xcraw{f}")
                xcraw.append(dst)
            else:
                dst = lp.tile([128, N], BF16, name=f"zt{f - 4}", tag=f"zt{f - 4}")
                zt.append(dst)
            for n0, nl in NC2:
                ps = pm.tile([128, nl], F32, name="inmm", tag="mm")
                _matsum(c, ps, [IL[k][f] for k in range(2)], xt, n0, nl)
                if f % 2 == 0:
                    nc.scalar.copy(dst[:, n0:n0 + nl], ps[:, :])
                else:
                    nc.vector.tensor_copy(dst[:, n0:n0 + nl], ps[:, :])
        # conv + silu -> xcs (f32r)
        for g in range(4):
            xcc = scrB(g, [128, N], F32, f"xcc{g}")
            nc.vector.tensor_scalar(xcc[:, :], xcraw[g][:, :], cw1[:, g:g + 1],
                                    cb[:, g:g + 1], AL.mult, AL.add)
            nc.vector.scalar_tensor_tensor(xcc[:, 1:], xcraw[g][:, :N - 1],
                                           cw0[:, g:g + 1], xcc[:, 1:],
                                           AL.mult, AL.add)
            e = scrA(g, [128, N], F32, f"cve{g}")
            nc.scalar.activation(e[:, :], xcc[:, :], AF.Exp, scale=-1.0)
            nc.vector.tensor_scalar_add(e[:, :], e[:, :], 1.0)
            nc.vector.reciprocal(e[:, :], e[:, :])
            o = lp.tile([128, N], F32R, name=f"xcs{g}", tag=f"xcs{g}")
            nc.vector.tensor_mul(o[:, :], xcc[:, :], e[:, :])
            xcs.append(o)

    # x_proj + dt
    dtT = []
    with c.tc.tile_pool(name=f"w2_{l}", bufs=1) as wp2:
        XP = _load_tiles_dir(c, wp2, "xp", l, tag="xpl")  # 4 x [128,48]
        dtin = lp.tile([16, N], F32R, name="dtin", tag="dtin")
        bcrows = lp.tile([32, N], BF16, name="bcrows", tag="bcrows")
        for n0, nl in NC2:
            ps = pm.tile([32, nl], F32, name="xpmm", tag="mm")
            _matsum(c, ps, [XP[k][0][:, DTR:] for k in range(4)], xcs, n0, nl)
            nc.scalar.copy(bcrows[:, n0:n0 + nl], ps[:, :])
            ps2 = pm.tile([16, nl], F32, name="xpmm2", tag="mm")
            _matsum(c, ps2, [XP[k][0][:, :DTR] for k in range(4)], xcs, n0, nl)
            nc.scalar.copy(dtin[:, n0:n0 + nl], ps2[:, :])
        bc_dram = c.dp.tile([32, N], BF16, name=f"bcd{l}", tag="bc_dram")
        nc.sync.dma_start(bc_dram[:, :], bcrows[:, :])
        DTW = _load_tiles_dir(c, wp2, "dt", l, tag="dtl")  # 1 x [16,512] in 4 col chunks
        dtb = _priv_cols(c, lp, l, 3)
        for g in range(4):
            u = scrA(g, [128, N], F32, f"dtu{g}")
            for n0, nl in NC2:
                ps = pm.tile([128, nl], F32, name="dtmm", tag="mm")
                nc.tensor.matmul(ps[:, :], DTW[0][g][:, :], dtin[:, n0:n0 + nl],
                                 start=True, stop=True)
                nc.scalar.activation(u[:, n0:n0 + nl], ps[:, :], AF.Exp,
                                     bias=dtb[:, g:g + 1])
            dt_ = lp.tile([128, N], F32, name=f"dtT{g}", tag=f"dtT{g}")
            nc.scalar.activation(dt_[:, :], u[:, :], AF.Ln, bias=1.0)
            dtT.append(dt_)
    wT = []
    for g in range(4):
        w_ = lp.tile([128, N], BF16, name=f"wT{g}", tag=f"wT{g}")
        nc.vector.tensor_mul(w_[:, :], dtT[g][:, :], xcs[g][:, :].bitcast(F32))
        wT.append(w_)

    # ---- scan: 16 states s, grouped in pairs for the reduction tree
    ytile = [None] * 4
    for grp in range(8):
        tmp2 = [scrA(g, [128, 2 * N], BF16, f"tmp2_{g}") for g in range(4)]
        for si in range(2):
            s = grp * 2 + si
            Bb = rp.tile([128, N], BF16, name="Bb", tag="Bb", bufs=2)
            nc.sync.dma_start(Bb[:, :],
                                bc_dram[s:s + 1, :].broadcast_to([128, N]))
            Cb = rp.tile([128, N], BF16, name="Cb", tag="Cb", bufs=2)
            nc.sync.dma_start(Cb[:, :],
                                bc_dram[16 + s:17 + s, :].broadcast_to([128, N]))
            for g in range(4):
                da = pa.tile([128, N], F32, name="dA", tag="dA")
                nc.scalar.activation(da[:, :], dtT[g][:, :], AF.Exp,
                                     scale=float(-(s + 1)))
                dbx = rp.tile([128, N], BF16, name="dbx", tag="dbx", bufs=2)
                nc.vector.tensor_mul(dbx[:, :], wT[g][:, :], Bb[:, :])
                h = rp.tile([128, N], BF16, name="h", tag="h", bufs=2)
                nc.vector.tensor_tensor_scan(h[:, :], da[:, :], dbx[:, :], 0.0,
                                             AL.mult, AL.add)
                nc.vector.tensor_mul(tmp2[g][:, si * N:(si + 1) * N],
                                     h[:, :], Cb[:, :])
        for g in range(4):
            if grp == 0:
                y_ = scrB(g, [128, N], F32, f"y{g}")
                nc.vector.tensor_add(y_[:, :], tmp2[g][:, 0:N],
                                     tmp2[g][:, N:2 * N])
                ytile[g] = y_
            else:
                t01 = rp.tile([128, N], BF16, name="t01", tag="t01", bufs=2)
                nc.vector.tensor_add(t01[:, :], tmp2[g][:, 0:N],
                                     tmp2[g][:, N:2 * N])
                nc.vector.tensor_add(ytile[g][:, :], ytile[g][:, :], t01[:, :])

    # ---- gating
    Dcol = _priv_cols(c, lp, l, 4)
    ym = []
    for g in range(4):
        yg = scrA(g, [128, N], F32, f"yg{g}")
        nc.vector.scalar_tensor_tensor(yg[:, :], xcs[g][:, :].bitcast(F32),
                                       Dcol[:, g:g + 1], ytile[g][:, :],
                                       AL.mult, AL.add)
        e2b = lp.tile([128, N], F32, name=f"gze{g}", tag=f"dtT{g}")
        nc.scalar.activation(e2b[:, :], zt[g][:, :], AF.Exp, scale=-1.0)
        nc.vector.tensor_scalar_add(e2b[:, :], e2b[:, :], 1.0)
        nc.vector.reciprocal(e2b[:, :], e2b[:, :])
        zr = scrB(g, [128, N], F32, f"zr{g}")
        nc.vector.tensor_mul(zr[:, :], zt[g][:, :], e2b[:, :])
        o = lp.tile([128, N], F32R, name=f"ym{g}", tag=f"xcs{g}")
        nc.vector.tensor_mul(o[:, :], yg[:, :], zr[:, :])
        ym.append(o)

    # ---- out_proj + exchange + LN1 + FFN + LN2
    with c.tc.tile_pool(name=f"w3_{l}", bufs=1) as wp3:
        OL = _load_tiles_dir(c, wp3, "out", l, tag="outl")
        fT = []
        for mi in range(2):
            t_ = lp.tile([128, N], F32, name=f"fT{mi}", tag=f"fT{mi}")
            fT.append(t_)
            for n0, nl in NC2:
                ps = pm.tile([128, nl], F32, name="opmm", tag="mm")
                _matsum(c, ps, [OL[k][mi] for k in range(4)], ym, n0, nl)
                nc.scalar.copy(t_[:, n0:n0 + nl], ps[:, :])

        fdram = c.dp.tile([256, N], F32, name=f"fd{l}", tag="fdram")
        sdram = c.dp.tile([256, N], F32, name=f"sd{l}", tag="sdram")
        for mi in range(2):
            nc.sync.dma_start(fdram[mi * 128:(mi + 1) * 128, :], fT[mi][:, :])
        nc.gpsimd.collective_compute("AllReduce", AL.add, replica_groups=PAIRS,
                                     ins=[fdram.opt()], outs=[sdram.opt()])
        xnew = []
        for mi in range(2):
            s_ = scrA(mi, [128, N], F32, f"exs{mi}")
            nc.sync.dma_start(s_[:, :], sdram[mi * 128:(mi + 1) * 128, :])
            nc.vector.tensor_sub(s_[:, :], s_[:, :], fT[mi][:, :])
            dr = scrA(mi + 2, [128, N], F32, f"exd{mi}")
            nc.scalar.copy(dr[:, :], s_[:, ::-1])
            a1 = scrB(mi, [128, N], F32, f"exa{mi}")
            nc.vector.tensor_add(a1[:, :], xt[mi][:, :].bitcast(F32),
                                 fT[mi][:, :])
            xv = lp.tile([128, N], F32R, name=f"xnew{mi}", tag=f"wT{mi}")
            nc.vector.tensor_add(xv[:, :], a1[:, :], dr[:, :])
            xnew.append(xv)
        n1w = _load_cols(c, lp, f"n1w_{l}")
        n1b = _load_cols(c, lp, f"n1b_{l}")
        xln = _layer_norm(c, rp, xnew, n1w, n1b, lp, f"xln{l}_")

        F1 = _load_tiles_g(c, wp3, f"f1_{l}", tag="f1l")
        F2 = _load_tiles_g(c, wp3, f"f2_{l}", tag="f2l")
        f1b = _load_cols(c, lp, f"f1b_{l}")
        f2b = _load_cols(c, lp, f"f2b_{l}")
        h1 = []
        for mf in range(2):
            t_ = lp.tile([128, N], F32R, name=f"ffh{mf}", tag=f"xcs{mf}")
            h1.append(t_)
            for n0, nl in NC2:
                ps = pm.tile([128, nl], F32, name="f1mm", tag="mm")
                _matsum(c, ps, [F1[k][mf] for k in range(2)], xln, n0, nl)
                nc.scalar.activation(t_[:, n0:n0 + nl], ps[:, :],
                                     AF.Gelu,
                                     bias=f1b[:, mf:mf + 1])
        xe2 = []
        for mi in range(2):
            y2 = scrA(mi, [128, N], F32, f"ffy{mi}")
            for n0, nl in NC2:
                ps = pm.tile([128, nl], F32, name="f2mm", tag="mm")
                _matsum(c, ps, [F2[k][mi] for k in range(2)], h1, n0, nl)
                nc.scalar.activation(y2[:, n0:n0 + nl], ps[:, :], AF.Identity,
                                     bias=f2b[:, mi:mi + 1])
            xv = lp.tile([128, N], F32R, name=f"xe2{mi}", tag=f"xcs{mi + 2}")
            nc.vector.tensor_add(xv[:, :],
                                 xln[mi][:, :].bitcast(F32), y2[:, :])
            xe2.append(xv)
        n2w = _load_cols(c, lp, f"n2w_{l}")
        n2b = _load_cols(c, lp, f"n2b_{l}")
        xout = _layer_norm(c, rp, xe2, n2w, n2b, c.gp,
                           "xtB" if l % 2 == 0 else "xtA")
    return xout


# ---------------------------------------------------------------- dispatch
def _get_program():
    if "prog" not in _CACHE:
        _CACHE["prog"] = _build()
    return _CACHE["prog"]


def _get_runner():
    if "runner" in _CACHE:
        return _CACHE["runner"]
    nc = _get_program()
    import jax
    from jax.sharding import Mesh, PartitionSpec
    from jax.experimental.shard_map import shard_map
    from concourse import bass2jax as b2j

    b2j.install_neuronx_cc_hook()
    n_cores = 8
    partition_name = (nc.partition_id_tensor.name
                      if nc.partition_id_tensor else None)
    in_names, out_names, out_avals, zero_spec = [], [], [], []
    for alloc in nc.m.functions[0].allocations:
        if not isinstance(alloc, mybir.MemoryLocationSet):
            continue
        name = alloc.memorylocations[0].name
        if alloc.kind == "ExternalInput":
            if name != partition_name:
                in_names.append(name)
        elif alloc.kind == "ExternalOutput":
            shape = tuple(alloc.tensor_shape)
            dtype = mybir.dt.np(alloc.dtype)
            out_names.append(name)
            out_avals.append(jax.core.ShapedArray(shape, dtype))
            zero_spec.append((shape, dtype))
    n_params = len(in_names)
    n_outs = len(out_avals)
    all_names = list(in_names) + list(out_names)
    if partition_name is not None:
        all_names.append(partition_name)
    donate = tuple(range(n_params, n_params + n_outs))

    def _body(*args):
        operands = list(args)
        if partition_name is not None:
            operands.append(b2j.partition_id_tensor())
        outs = b2j._bass_exec_p.bind(
            *operands, out_avals=tuple(out_avals), in_names=tuple(all_names),
            out_names=tuple(out_names), lowering_input_output_aliases=(),
            sim_require_finite=True, sim_require_nnan=True, nc=nc)
        return tuple(outs)

    devices = jax.devices()[:n_cores]
    mesh = Mesh(np.asarray(devices), ("core",))
    in_specs = (PartitionSpec("core"),) * (n_params + n_outs)
    out_specs = (PartitionSpec("core"),) * n_outs
    jitted = jax.jit(
        shard_map(_body, mesh=mesh, in_specs=in_specs, out_specs=out_specs,
                  check_rep=False),
        donate_argnums=donate, keep_unused=True)
    runner = {"jitted": jitted, "compiled": None, "in_names": in_names,
              "out_names": out_names, "out_avals": out_avals,
              "zero_spec": zero_spec}
    _CACHE["runner"] = runner
    return runner


def _dispatch(in_maps):
    """One full dispatch: h2d of per-core inputs, exec, d2h of outputs."""
    r = _get_runner()
    n_cores = 8
    concat_in = [
        np.concatenate([np.asarray(in_maps[c][name]) for c in range(n_cores)],
                       axis=0)
        for name in r["in_names"]]
    concat_zeros = [np.zeros((n_cores * s[0], *s[1:]), d)
                    for s, d in r["zero_spec"]]
    if r["compiled"] is None:
        r["compiled"] = r["jitted"].lower(*concat_in, *concat_zeros).compile()
    out_arrs = r["compiled"](*concat_in, *concat_zeros)
    return [
        {name: np.asarray(out_arrs[i]).reshape(
            n_cores, *r["out_avals"][i].shape)[c]
         for i, name in enumerate(r["out_names"])}
        for c in range(n_cores)]


def kernel(**inputs):
    in_maps = make_core_inputs_all(inputs)
    res = _dispatch(in_maps)
    out = np.empty((B, H, N, 1), np.float32)
    for b in range(B):
        out[b, :, :, 0] = res[2 * b]["pred"].astype(np.float32)
    return out


if __name__ == "__main__":
    print("building program...")
    _get_program()
    print("built ok")
